# revision 15
# baseline (speedup 1.0000x reference)
"""Trainium2 Bass kernel for nn_Decoder (seq2seq LSTM decoder w/ attention + copy).

Strategy: teacher forcing means only the LSTM recurrences are sequential.
 - Encoder fwd/bwd scans interleaved on each core (replicated), fp16 weights,
   orientation-A matvecs (weights stationary) -> gates in [128, nchunk] layout.
 - Decoder scan likewise.
 - Everything else batched: input projections, attention (Q/scores/softmax/ctx),
   output projection, vocab matmul (sharded over V across 8 cores), p_copy via
   onehot matmul, softmax-sum AllReduce, combine, DMA out.
Host: weight layout prep (permute gates, transpose, fp16 cast), final concat +
log-prob reduction.
"""

import numpy as np

import concourse.bass as bass
from concourse import bacc
import concourse.mybir as mybir
import concourse.tile as tile
from concourse.bass import IndirectOffsetOnAxis
from concourse.bass_utils import run_bass_kernel_spmd
from concourse.masks import make_identity

FP = mybir.dt.float32
HP = mybir.dt.float16
I32 = mybir.dt.int32

E = 256
H2 = 512
V = 32000
S_FULL = 512
T_FULL = 128
NCORES = 8
SOS = 1
NEG = -1.0e30
GW = 1024  # vocab psum group width (2 banks of fp32)

AX = mybir.AxisListType
ALU = mybir.AluOpType
ACTF = mybir.ActivationFunctionType


def _perm_ifog(n4: int) -> np.ndarray:
    """Row permutation taking torch gate order [i,f,g,o] -> [i,f,o,g]."""
    h = n4 // 4
    return np.concatenate(
        [np.arange(0, h), np.arange(h, 2 * h), np.arange(3 * h, 4 * h),
         np.arange(2 * h, 3 * h)]
    )


def _vgroups(Vs):
    """PSUM-bank-aligned grouping of the vocab slice: groups of <=1024 cols,
    each split into <=512-wide chunks (one fp32 psum bank per chunk)."""
    groups = []
    g0 = 0
    while g0 < Vs:
        gw = min(GW, Vs - g0)
        chunks = []
        o = 0
        while o < gw:
            w = min(512, gw - o)
            chunks.append((o, w))
            o += w
        groups.append((g0, gw, chunks))
        g0 += gw
    return groups


def build_program(S, T, Vs, n_cores, use_collective, debug=False):
    """Builds the SPMD bass program. Returns (nc, dram_map, debug_names)."""
    nc = bacc.Bacc(
        "TRN2", target_bir_lowering=False, debug=False, num_devices=n_cores
    )
    SC = S // 128
    NG = len(_vgroups(Vs))

    d = {}

    def din(name, shape, dt):
        d[name] = nc.dram_tensor(name, list(shape), dt, kind="ExternalInput")
        return d[name]

    def dout(name, shape, dt):
        d[name] = nc.dram_tensor(name, list(shape), dt, kind="ExternalOutput")
        return d[name]

    din("tokens_i32", (128, SC), I32)
    din("trgprev_i32", (128, 1), I32)
    din("tokloc_f32", (128, SC, NG), FP)   # tokens - core*Vs - g*GW
    din("maskvec", (1, S), FP)
    Vtot = Vs * n_cores
    din("enc_emb", (Vtot, E), FP)
    din("dec_emb", (Vtot, H2), FP)
    din("wihT_f", (2, 128, 4 * E), FP)
    din("wihT_b", (2, 128, 4 * E), FP)
    din("whhT_f", (2, 128, 4 * E), HP)
    din("whhT_b", (2, 128, 4 * E), HP)
    din("bias_f", (128, 8), FP)
    din("bias_b", (128, 8), FP)
    din("dwihT_e2", (4, 128, 4 * H2), HP)
    din("dwihT_eh", (4, 128, 4 * H2), HP)
    din("dwhhT", (4, 128, 4 * H2), HP)
    din("dbias", (128, 16), FP)
    din("attn_wT", (4, 128, H2), HP)
    din("attn_b", (128, 4), FP)
    din("w_wT", (8, 128, 2 * H2), HP)
    din("w_b", (128, 8), FP)
    din("linwT", (8, 128, Vs), HP)
    din("linb", (1, Vs), HP)
    din("copywT", (128, 12), HP)
    din("copyb_col", (128, 1), FP)
    din("copyb_row", (1, 1), FP)
    dout("p_out", (128, Vs), FP)

    dbg_names = []

    def ddbg(name, shape, dt=FP):
        if debug:
            dout(name, shape, dt)
            dbg_names.append(name)

    ddbg("dbg_hsf", (128, 2, S), HP)
    ddbg("dbg_hsb", (128, 2, S), HP)
    ddbg("dbg_dh", (128, 4, T), HP)
    ddbg("dbg_aw", (128, S))
    ddbg("dbg_scores", (128, S))
    ddbg("dbg_ctx", (128, 4, 128), HP)
    ddbg("dbg_ohc", (128, 8, 128), HP)
    ddbg("dbg_xwf", (128, 8, S))
    ddbg("dbg_xd", (128, 16, T))
    ddbg("dbg_sexp", (128, 1))
    ddbg("dbg_pg", (128, 1))

    with tile.TileContext(nc) as tc:
        _emit(nc, tc, d, S, T, Vs, SC, n_cores, use_collective, debug)
    nc.compile()
    return nc, d, dbg_names


def _emit(nc, tc, d, S, T, Vs, SC, n_cores, use_collective, debug):
    from contextlib import ExitStack

    vgroups = _vgroups(Vs)
    ctx = ExitStack()
    with ctx:
        sing = ctx.enter_context(tc.tile_pool(name="sing", bufs=1))
        spool = ctx.enter_context(tc.tile_pool(name="scratch", bufs=3))
        dram = ctx.enter_context(tc.tile_pool(name="dramp", bufs=1, space="DRAM"))
        pxw = ExitStack()
        xwpool = pxw.enter_context(tc.tile_pool(name="xwpool", bufs=1))
        pearly = ExitStack()
        early = pearly.enter_context(tc.tile_pool(name="early", bufs=1))
        wpool = pearly.enter_context(tc.tile_pool(name="wstream", bufs=2))
        ppool = pearly.enter_context(
            tc.tile_pool(name="psA", bufs=2, space="PSUM"))

        def st(tag, shape, dt):
            return sing.tile(shape, dt, tag=tag, name=tag)

        def ste(tag, shape, dt):
            return early.tile(shape, dt, tag=tag, name=tag)

        # ---------------- constants / small loads ----------------
        ident_f = st("ident_f", [128, 128], FP)
        make_identity(nc, ident_f[:])
        ident_h = st("ident_h", [128, 128], HP)
        make_identity(nc, ident_h[:])
        ones_h = st("ones_h", [1, 128], HP)
        nc.vector.memset(ones_h[:], 1.0)

        def load(tag, dname, shape, dt, rearr=None):
            t = st(tag, shape, dt)
            src = d[dname].ap()
            if rearr:
                src = src.rearrange(rearr)
            nc.sync.dma_start(out=t[:], in_=src)
            return t

        tok_sb = load("tok_sb", "tokens_i32", [128, SC], I32)
        trg_sb = load("trg_sb", "trgprev_i32", [128, 1], I32)
        NGq = len(vgroups)
        tokloc = load("tokloc", "tokloc_f32", [128, SC, NGq], FP)
        maskrow = load("maskrow", "maskvec", [1, S], FP)
        bias_f = load("bias_f", "bias_f", [128, 8], FP)
        bias_b = load("bias_b", "bias_b", [128, 8], FP)
        dbias = load("dbias", "dbias", [128, 16], FP)
        attn_b = load("attn_b", "attn_b", [128, 4], FP)
        w_b = load("w_b", "w_b", [128, 8], FP)
        copywT = load("copywT", "copywT", [128, 12], HP)
        copyb_col = load("copyb_col", "copyb_col", [128, 1], FP)
        copyb_row = load("copyb_row", "copyb_row", [1, 1], FP)
        linb = load("linb", "linb", [1, Vs], HP)
        whh_f = load("whh_f", "whhT_f", [128, 2, 4 * E], HP, "k p m -> p k m")
        whh_b = load("whh_b", "whhT_b", [128, 2, 4 * E], HP, "k p m -> p k m")

        # ---------------- embedding gathers + transposes ----------------
        xg = ste("xgather", [128, SC, E], FP)
        for c in range(SC):
            nc.gpsimd.indirect_dma_start(
                out=xg[:, c, :], out_offset=None, in_=d["enc_emb"].ap(),
                in_offset=IndirectOffsetOnAxis(ap=tok_sb[:, c:c + 1], axis=0),
            )
        xT = ste("xT", [128, 2, S], FP)
        for c in range(SC):
            for kc in range(2):
                pt = ppool.tile([128, 128], FP, tag="tp", name="tp")
                nc.tensor.transpose(
                    pt[:], xg[:, c, kc * 128:(kc + 1) * 128], ident_f[:])
                nc.scalar.activation(
                    out=xT[:, kc, c * 128:(c + 1) * 128], in_=pt[:],
                    func=ACTF.Relu)
        e2g = st("e2g", [128, H2], FP)
        nc.gpsimd.indirect_dma_start(
            out=e2g[:, :], out_offset=None, in_=d["dec_emb"].ap(),
            in_offset=IndirectOffsetOnAxis(ap=trg_sb[:, 0:1], axis=0),
        )
        e2T = st("e2T", [128, 4, 128], HP)
        for kc in range(4):
            pt = ppool.tile([128, 128], FP, tag="tp", name="tp")
            nc.tensor.transpose(
                pt[:], e2g[:, kc * 128:(kc + 1) * 128], ident_f[:])
            nc.scalar.activation(out=e2T[:, kc, :], in_=pt[:], func=ACTF.Relu)

        # ---------------- encoder input projections (batched) ----------------
        xw = {}
        for dirn, wname, bias in (("f", "wihT_f", bias_f), ("b", "wihT_b", bias_b)):
            xw[dirn] = xwpool.tile([128, 8, S], FP, tag=f"xw_{dirn}", name=f"xw_{dirn}")
            wt = wpool.tile([128, 2, 4 * E], FP, tag="wih", name="wih")
            nc.sync.dma_start(
                out=wt[:], in_=d[wname].ap().rearrange("k p m -> p k m"))
            for j in range(8):
                ps = ppool.tile([128, S], FP, tag="xwps", name="xwps")
                for kc in range(2):
                    nc.tensor.matmul(
                        ps[:], wt[:, kc, j * 128:(j + 1) * 128], xT[:, kc, :],
                        start=(kc == 0), stop=(kc == 1))
                nc.vector.tensor_scalar(
                    out=xw[dirn][:, j, :], in0=ps[:],
                    scalar1=bias[:, j:j + 1], scalar2=None, op0=ALU.add)
        if debug:
            nc.sync.dma_start(out=d["dbg_xwf"].ap(), in_=xw["f"][:])

        # ---------------- encoder scans (fwd/bwd interleaved) ----------------
        pearly.close()
        pscan = ExitStack()
        gpool_f = pscan.enter_context(
            tc.tile_pool(name="gates_f", bufs=2, space="PSUM"))
        gpool_b = pscan.enter_context(
            tc.tile_pool(name="gates_b", bufs=2, space="PSUM"))
        hs = {"f": st("hs_f", [128, 2, S], HP), "b": st("hs_b", [128, 2, S], HP)}
        h0 = st("h0", [128, 2], HP)
        nc.vector.memset(h0[:], 0.0)
        c_st = {"f": st("c_f", [128, 2], FP), "b": st("c_b", [128, 2], FP)}
        nc.vector.memset(c_st["f"][:], 0.0)
        nc.vector.memset(c_st["b"][:], 0.0)

        def lstm_step(dirn, whh_sb, xw_sb, gp, t):
            store = t if dirn == "f" else S - 1 - t
            prev = None if t == 0 else (t - 1 if dirn == "f" else S - t)
            ps = gp.tile([128, 8], FP, tag=f"g{dirn}", name=f"g{dirn}")
            for j in range(8):
                for kc in range(2):
                    rhs = (h0[:, kc:kc + 1] if prev is None
                           else hs[dirn][:, kc, prev:prev + 1])
                    nc.tensor.matmul(
                        ps[:, j:j + 1],
                        whh_sb[:, kc, j * 128:(j + 1) * 128], rhs,
                        start=(kc == 0), stop=(kc == 1))
            xgt = spool.tile([128, 8], FP, tag=f"xg{dirn}", name=f"xg{dirn}")
            nc.vector.tensor_tensor(
                out=xgt[:], in0=ps[:], in1=xw_sb[:, :, store], op=ALU.add)
            sg = spool.tile([128, 6], FP, tag=f"sg{dirn}", name=f"sg{dirn}")
            nc.scalar.activation(out=sg[:], in_=xgt[:, 0:6], func=ACTF.Sigmoid)
            tg = spool.tile([128, 2], FP, tag=f"tg{dirn}", name=f"tg{dirn}")
            nc.scalar.activation(out=tg[:], in_=xgt[:, 6:8], func=ACTF.Tanh)
            t1 = spool.tile([128, 2], FP, tag=f"t1{dirn}", name=f"t1{dirn}")
            nc.vector.tensor_tensor(
                out=t1[:], in0=sg[:, 2:4], in1=c_st[dirn][:], op=ALU.mult)
            t2 = spool.tile([128, 2], FP, tag=f"t2{dirn}", name=f"t2{dirn}")
            nc.vector.tensor_tensor(
                out=t2[:], in0=sg[:, 0:2], in1=tg[:], op=ALU.mult)
            nc.vector.tensor_tensor(
                out=c_st[dirn][:], in0=t1[:], in1=t2[:], op=ALU.add)
            tc_ = spool.tile([128, 2], FP, tag=f"tc{dirn}", name=f"tc{dirn}")
            nc.scalar.activation(out=tc_[:], in_=c_st[dirn][:], func=ACTF.Tanh)
            nc.vector.tensor_tensor(
                out=hs[dirn][:, :, store], in0=sg[:, 4:6], in1=tc_[:],
                op=ALU.mult)

        for t in range(S):
            lstm_step("f", whh_f, xw["f"], gpool_f, t)
            lstm_step("b", whh_b, xw["b"], gpool_b, t)
        if debug:
            nc.sync.dma_start(out=d["dbg_hsf"].ap(), in_=hs["f"][:])
            nc.sync.dma_start(out=d["dbg_hsb"].ap(), in_=hs["b"][:])

        def ehid_chunk(kc):
            if kc < 2:
                return hs["f"][:, kc, S - 1:S]
            return hs["b"][:, kc - 2, 0:1]

        # ---------------- decoder input precompute ----------------
        pscan.close()
        pxw.close()
        late = ctx.enter_context(tc.tile_pool(name="late", bufs=1))
        wpool2 = ctx.enter_context(tc.tile_pool(name="wstream2", bufs=2))
        sm1 = ctx.enter_context(tc.tile_pool(name="sm1", bufs=1))
        ohpool = ctx.enter_context(tc.tile_pool(name="ohpool", bufs=2))

        def stl(tag, shape, dt):
            return late.tile(shape, dt, tag=tag, name=tag)

        dwhh = stl("dwhh", [128, 4, 4 * H2], HP)
        nc.sync.dma_start(
            out=dwhh[:], in_=d["dwhhT"].ap().rearrange("k p m -> p k m"))
        linw = stl("linw", [128, 8, Vs], HP)
        nc.sync.dma_start(
            out=linw[:], in_=d["linwT"].ap().rearrange("k p m -> p k m"))
        iota = stl("iota", [128, GW], FP)
        nc.gpsimd.iota(
            out=iota[:], pattern=[[1, GW]], base=0, channel_multiplier=0,
            allow_small_or_imprecise_dtypes=True,
        )
        pb2 = ExitStack()
        bigp = pb2.enter_context(
            tc.tile_pool(name="psB2", bufs=1, space="PSUM"))
        ppool2 = pb2.enter_context(
            tc.tile_pool(name="psB2s", bufs=1, space="PSUM"))
        xd_ps = bigp.tile([128, 16, T], FP, tag="xdps", name="xdps")
        we2 = wpool2.tile([128, 4, 4 * H2], HP, tag="wdec", name="we2")
        nc.sync.dma_start(
            out=we2[:], in_=d["dwihT_e2"].ap().rearrange("k p m -> p k m"))
        for j in range(16):
            for kc in range(4):
                nc.tensor.matmul(
                    xd_ps[:, j, :], we2[:, kc, j * 128:(j + 1) * 128],
                    e2T[:, kc, :], start=(kc == 0), stop=(kc == 3))
        cvec_ps = ppool2.tile([128, 16], FP, tag="cvps", name="cvps")
        weh = wpool2.tile([128, 4, 4 * H2], HP, tag="wdec", name="weh")
        nc.sync.dma_start(
            out=weh[:], in_=d["dwihT_eh"].ap().rearrange("k p m -> p k m"))
        for j in range(16):
            for kc in range(4):
                nc.tensor.matmul(
                    cvec_ps[:, j:j + 1], weh[:, kc, j * 128:(j + 1) * 128],
                    ehid_chunk(kc), start=(kc == 0), stop=(kc == 3))
        cvec = stl("cvec", [128, 16], FP)
        nc.vector.tensor_tensor(
            out=cvec[:], in0=cvec_ps[:], in1=dbias[:], op=ALU.add)
        xd = stl("xd", [128, 16, T], FP)
        for j in range(16):
            nc.vector.tensor_scalar(
                out=xd[:, j, :], in0=xd_ps[:, j, :],
                scalar1=cvec[:, j:j + 1], scalar2=None, op0=ALU.add)
        if debug:
            nc.sync.dma_start(out=d["dbg_xd"].ap(), in_=xd[:])

        # ---------------- decoder scan ----------------
        pb2.close()
        pdec = ExitStack()
        gpool_d = pdec.enter_context(
            tc.tile_pool(name="gates_d", bufs=2, space="PSUM"))
        dh = st("dh", [128, 4, T], HP)
        dc = st("dc", [128, 4], FP)
        nc.vector.memset(dc[:], 0.0)
        for t in range(T):
            ps = gpool_d.tile([128, 16], FP, tag="gd", name="gd")
            for j in range(16):
                for kc in range(4):
                    rhs = ehid_chunk(kc) if t == 0 else dh[:, kc, t - 1:t]
                    nc.tensor.matmul(
                        ps[:, j:j + 1],
                        dwhh[:, kc, j * 128:(j + 1) * 128], rhs,
                        start=(kc == 0), stop=(kc == 3))
            xgt = spool.tile([128, 16], FP, tag="xgd", name="xgd")
            nc.vector.tensor_tensor(
                out=xgt[:], in0=ps[:], in1=xd[:, :, t], op=ALU.add)
            sg = spool.tile([128, 12], FP, tag="sgd", name="sgd")
            nc.scalar.activation(out=sg[:], in_=xgt[:, 0:12], func=ACTF.Sigmoid)
            tg = spool.tile([128, 4], FP, tag="tgd", name="tgd")
            nc.scalar.activation(out=tg[:], in_=xgt[:, 12:16], func=ACTF.Tanh)
            t1 = spool.tile([128, 4], FP, tag="t1d", name="t1d")
            nc.vector.tensor_tensor(
                out=t1[:], in0=sg[:, 4:8], in1=dc[:], op=ALU.mult)
            t2 = spool.tile([128, 4], FP, tag="t2d", name="t2d")
            nc.vector.tensor_tensor(
                out=t2[:], in0=sg[:, 0:4], in1=tg[:], op=ALU.mult)
            nc.vector.tensor_tensor(out=dc[:], in0=t1[:], in1=t2[:], op=ALU.add)
            tc_ = spool.tile([128, 4], FP, tag="tcd", name="tcd")
            nc.scalar.activation(out=tc_[:], in_=dc[:], func=ACTF.Tanh)
            nc.vector.tensor_tensor(
                out=dh[:, :, t], in0=sg[:, 8:12], in1=tc_[:], op=ALU.mult)
        if debug:
            nc.sync.dma_start(out=d["dbg_dh"].ap(), in_=dh[:])

        # ---------------- batched attention ----------------
        pdec.close()
        patt = ExitStack()
        ppool = patt.enter_context(
            tc.tile_pool(name="psD", bufs=2, space="PSUM"))
        ppd1 = patt.enter_context(
            tc.tile_pool(name="psD1", bufs=1, space="PSUM"))
        qt = st("qt", [128, 4, T], HP)
        wat = wpool2.tile([128, 4, H2], HP, tag="wdec", name="wat")
        nc.sync.dma_start(
            out=wat[:], in_=d["attn_wT"].ap().rearrange("k p m -> p k m"))
        for jq in range(4):
            ps = ppool.tile([128, T], FP, tag="mm128", name="mm128")
            for kc in range(4):
                nc.tensor.matmul(
                    ps[:], wat[:, kc, jq * 128:(jq + 1) * 128], dh[:, kc, :],
                    start=(kc == 0), stop=(kc == 3))
            nc.vector.tensor_scalar(
                out=qt[:, jq, :], in0=ps[:], scalar1=attn_b[:, jq:jq + 1],
                scalar2=None, op0=ALU.add)

        def encT_chunk(kc):
            return hs["f"][:, kc, :] if kc < 2 else hs["b"][:, kc - 2, :]

        sc_ps = ppd1.tile([128, S], FP, tag="scps", name="scps")
        for kc in range(4):
            nc.tensor.matmul(
                sc_ps[:], qt[:, kc, :], encT_chunk(kc),
                start=(kc == 0), stop=(kc == 3))
        mb128 = sm1.tile([128, S], FP, tag="mb128", name="mb128")
        mv_ap = d["maskvec"].ap()
        nc.sync.dma_start(
            out=mb128[:],
            in_=bass.AP(tensor=mv_ap.tensor, offset=mv_ap.offset,
                        ap=[[0, 128]] + [list(p) for p in mv_ap.ap[1:]]))
        scores = sm1.tile([128, S], FP, tag="scores", name="scores")
        nc.vector.tensor_tensor(
            out=scores[:], in0=sc_ps[:], in1=mb128[:], op=ALU.add)
        if debug:
            nc.sync.dma_start(out=d["dbg_scores"].ap(), in_=scores[:])
        rmax = sm1.tile([128, 1], FP, tag="rmax", name="rmax")
        nc.vector.tensor_reduce(out=rmax[:], in_=scores[:], axis=AX.X, op=ALU.max)
        nmax = sm1.tile([128, 1], FP, tag="nmax", name="nmax")
        nc.scalar.activation(out=nmax[:], in_=rmax[:], func=ACTF.Copy, scale=-1.0)
        aexp = sm1.tile([128, S], FP, tag="aexp", name="aexp")
        sexp = sm1.tile([128, 1], FP, tag="sexp", name="sexp")
        nc.scalar.activation(
            out=aexp[:], in_=scores[:], func=ACTF.Exp, bias=nmax[:, 0:1],
            accum_out=sexp[:, 0:1])
        rsum = sm1.tile([128, 1], FP, tag="rsum", name="rsum")
        nc.vector.reciprocal(out=rsum[:], in_=sexp[:])
        aw16 = sm1.tile([128, S], HP, tag="aw16", name="aw16")
        nc.vector.tensor_scalar(
            out=aw16[:], in0=aexp[:], scalar1=rsum[:, 0:1], scalar2=None,
            op0=ALU.mult)
        if debug:
            awdbg = sm1.tile([128, S], FP, tag="awdbg", name="awdbg")
            nc.vector.tensor_scalar(
                out=awdbg[:], in0=aexp[:], scalar1=rsum[:, 0:1], scalar2=None,
                op0=ALU.mult)
            nc.sync.dma_start(out=d["dbg_aw"].ap(), in_=awdbg[:])

        awt = stl("awt", [128, SC, T], HP)
        for c in range(SC):
            pt = ppool.tile([128, 128], HP, tag="tph", name="tph")
            nc.tensor.transpose(
                pt[:], aw16[:, c * 128:(c + 1) * 128], ident_h[:])
            nc.scalar.copy(out=awt[:, c, :], in_=pt[:, 0:T])
        encs = stl("encs", [128, SC, 4, 128], HP)
        for c in range(SC):
            for kd in range(4):
                pt = ppool.tile([128, 128], HP, tag="tph", name="tph")
                nc.tensor.transpose(
                    pt[:], encT_chunk(kd)[:, c * 128:(c + 1) * 128], ident_h[:])
                nc.scalar.copy(out=encs[:, c, kd, :], in_=pt[:])
        ctx16 = stl("ctx16", [128, 4, T], HP)
        for kd in range(4):
            ps = ppool.tile([128, T], FP, tag="mm128", name="mm128")
            for c in range(SC):
                nc.tensor.matmul(
                    ps[:], encs[:, c, kd, :], awt[:, c, :],
                    start=(c == 0), stop=(c == SC - 1))
            nc.scalar.copy(out=ctx16[:, kd, :], in_=ps[:])
        if debug:
            nc.sync.dma_start(out=d["dbg_ctx"].ap(), in_=ctx16[:])

        def hcT_chunk(kc):
            return dh[:, kc, :] if kc < 4 else ctx16[:, kc - 4, :]

        # ---------------- output projection OUT_HC.T ----------------
        ohc = stl("ohc", [128, 8, T], HP)
        wwt = stl("wwt", [128, 8, 2 * H2], HP)
        nc.sync.dma_start(
            out=wwt[:], in_=d["w_wT"].ap().rearrange("k p m -> p k m"))
        for jo in range(8):
            ps = ppool.tile([128, T], FP, tag="mm128", name="mm128")
            for kc in range(8):
                nc.tensor.matmul(
                    ps[:], wwt[:, kc, jo * 128:(jo + 1) * 128],
                    hcT_chunk(kc), start=(kc == 0), stop=(kc == 7))
            nc.vector.tensor_scalar(
                out=ohc[:, jo, :], in0=ps[:], scalar1=w_b[:, jo:jo + 1],
                scalar2=None, op0=ALU.add)
        if debug:
            nc.sync.dma_start(out=d["dbg_ohc"].ap(), in_=ohc[:])

        # ---------------- p_gen ----------------
        def hcxT_chunk(kc):
            if kc < 4:
                return dh[:, kc, :]
            if kc < 8:
                return ctx16[:, kc - 4, :]
            return e2T[:, kc - 8, :]

        pg_ps = ppd1.tile([128, 1], FP, tag="pgps", name="pgps")
        for kc in range(12):
            nc.tensor.matmul(
                pg_ps[:], hcxT_chunk(kc), copywT[:, kc:kc + 1],
                start=(kc == 0), stop=(kc == 11))
        pgr_ps = ppd1.tile([1, T], FP, tag="pgrps", name="pgrps")
        for kc in range(12):
            nc.tensor.matmul(
                pgr_ps[:], copywT[:, kc:kc + 1], hcxT_chunk(kc),
                start=(kc == 0), stop=(kc == 11))
        pg_col = stl("pg_col", [128, 1], FP)
        nc.scalar.activation(
            out=pg_col[:], in_=pg_ps[:], func=ACTF.Sigmoid,
            bias=copyb_col[:, 0:1])
        pg_row = stl("pg_row", [1, T], FP)
        nc.scalar.activation(
            out=pg_row[:], in_=pgr_ps[:], func=ACTF.Sigmoid,
            bias=copyb_row[:, 0:1])
        ompg_row = stl("ompg_row", [1, T], HP)
        nc.scalar.activation(
            out=ompg_row[:], in_=pg_row[:], func=ACTF.Copy, scale=-1.0, bias=1.0)
        omb_ps = ppool.tile([128, T], FP, tag="mm128", name="mm128")
        nc.tensor.matmul(omb_ps[:], ones_h[:], ompg_row[:],
                         start=True, stop=True)
        awts = stl("awts", [128, SC, T], HP)
        for c in range(SC):
            nc.vector.tensor_tensor(
                out=awts[:, c, :], in0=awt[:, c, :],
                in1=omb_ps[:], op=ALU.mult)
        if debug:
            nc.sync.dma_start(out=d["dbg_pg"].ap(), in_=pg_col[:])

        # ---------------- vocab logits + exp + sums ----------------
        patt.close()
        pend = ExitStack()
        bigp = pend.enter_context(
            tc.tile_pool(name="psE", bufs=1, space="PSUM"))
        ebuf = stl("ebuf", [128, Vs], FP)
        nparts = sum(len(g[2]) for g in vgroups)
        sparts = stl("sparts", [128, nparts], FP)
        pi = 0
        for g0, gw, chunks in vgroups:
            ps = bigp.tile([128, 2, 512], FP, tag="lgps", name="lgps", bufs=2)
            for ci, (o, w) in enumerate(chunks):
                for kc in range(8):
                    nc.tensor.matmul(
                        ps[:, ci, 0:w], ohc[:, kc, :],
                        linw[:, kc, g0 + o:g0 + o + w],
                        start=(kc == 0), stop=False)
                nc.tensor.matmul(
                    ps[:, ci, 0:w], ones_h[:], linb[:, g0 + o:g0 + o + w],
                    start=False, stop=True)
                nc.scalar.activation(
                    out=ebuf[:, g0 + o:g0 + o + w], in_=ps[:, ci, 0:w],
                    func=ACTF.Exp, accum_out=sparts[:, pi:pi + 1])
                pi += 1
        sloc = stl("sloc", [128, 1], FP)
        nc.vector.tensor_reduce(
            out=sloc[:], in_=sparts[:], axis=AX.X, op=ALU.add)

        # ---------------- softmax-sum allreduce ----------------
        sg_t = stl("sg_t", [128, 1], FP)
        if use_collective and n_cores > 1:
            cin = dram.tile([128, 1], FP, tag="ccin", name="ccin")
            cout = dram.tile([128, 1], FP, tag="ccout", name="ccout")
            nc.sync.dma_start(out=cin[:], in_=sloc[:])
            nc.gpsimd.collective_compute(
                "AllReduce", ALU.add,
                replica_groups=[list(range(n_cores))],
                ins=[cin[:].opt()], outs=[cout[:].opt()],
            )
            nc.sync.dma_start(out=sg_t[:], in_=cout[:])
        else:
            nc.vector.tensor_copy(out=sg_t[:], in_=sloc[:])
        if debug:
            nc.sync.dma_start(out=d["dbg_sexp"].ap(), in_=sg_t[:])

        rg = stl("rg", [128, 1], FP)
        nc.vector.reciprocal(out=rg[:], in_=sg_t[:])
        pgs = stl("pgs", [128, 1], FP)
        nc.vector.tensor_tensor(out=pgs[:], in0=pg_col[:], in1=rg[:], op=ALU.mult)
        nc.vector.tensor_scalar(
            out=ebuf[:], in0=ebuf[:], scalar1=pgs[:, 0:1], scalar2=None,
            op0=ALU.mult)

        # ---------------- p_copy = AW.T(1-pg) @ onehot ----------------
        for gi, (g0, gw, chunks) in enumerate(vgroups):
            ps = bigp.tile([128, 2, 512], FP, tag="pcps", name="pcps", bufs=2)
            for c in range(SC):
                oh = ohpool.tile([128, GW], HP, tag="oh", name="oh")
                nc.gpsimd.tensor_scalar(
                    out=oh[:, 0:gw], in0=iota[:, 0:gw],
                    scalar1=tokloc[:, c, gi:gi + 1], scalar2=None,
                    op0=ALU.is_equal)
                for ci, (o, w) in enumerate(chunks):
                    nc.tensor.matmul(
                        ps[:, ci, 0:w], awts[:, c, :], oh[:, o:o + w],
                        start=(c == 0), stop=(c == SC - 1))
            for ci, (o, w) in enumerate(chunks):
                nc.vector.tensor_tensor(
                    out=ebuf[:, g0 + o:g0 + o + w],
                    in0=ebuf[:, g0 + o:g0 + o + w],
                    in1=ps[:, ci, 0:w], op=ALU.add)
        nc.sync.dma_start(out=d["p_out"].ap(), in_=ebuf[:])
        pend.close()


# ---------------------------------------------------------------------------
# Host-side preparation
# ---------------------------------------------------------------------------


def prep_inputs(inputs, n_cores=NCORES, S=S_FULL, T=T_FULL, Vtot=V):
    f32, f16 = np.float32, np.float16
    Vs = Vtot // n_cores
    SC = S // 128
    vgroups = _vgroups(Vs)
    NG = len(vgroups)

    tokens = np.asarray(inputs["tokens"]).astype(np.int64)
    trg = np.asarray(inputs["trg_seqs"]).astype(np.int64)
    trgprev = np.concatenate([[SOS], trg[:-1]]).astype(np.int32)

    perm_e = _perm_ifog(4 * E)
    perm_d = _perm_ifog(4 * H2)

    def kchunk(a):
        K, M = a.shape
        return np.ascontiguousarray(a.reshape(K // 128, 128, M))

    def colchunk(v, nch):
        return np.ascontiguousarray(v.reshape(nch, 128).T)

    def g(name):
        return np.asarray(inputs[name]).astype(f32)

    common = {}
    common["tokens_i32"] = np.ascontiguousarray(
        tokens.astype(np.int32).reshape(SC, 128).T)
    common["trgprev_i32"] = trgprev.reshape(1, T).T.copy()
    common["maskvec"] = np.where(tokens > 0, 0.0, NEG).astype(f32)[None, :]
    common["enc_emb"] = g("enc_emb")
    common["dec_emb"] = g("dec_emb")
    for dirn in ("f", "b"):
        wih = g(f"Wih_{dirn}")[perm_e]
        whh = g(f"Whh_{dirn}")[perm_e]
        bias = (g(f"bih_{dirn}") + g(f"bhh_{dirn}"))[perm_e]
        common[f"wihT_{dirn}"] = kchunk(wih.T.astype(f32))
        common[f"whhT_{dirn}"] = kchunk(whh.T.astype(f16))
        common[f"bias_{dirn}"] = colchunk(bias, 8)
    dwih = g("dWih")[perm_d]
    dwhh = g("dWhh")[perm_d]
    dbias = (g("dbih") + g("dbhh"))[perm_d]
    common["dwihT_e2"] = kchunk(dwih[:, :H2].T.astype(f16))
    common["dwihT_eh"] = kchunk(dwih[:, H2:].T.astype(f16))
    common["dwhhT"] = kchunk(dwhh.T.astype(f16))
    common["dbias"] = colchunk(dbias, 16)
    common["attn_wT"] = kchunk(g("attn_W").T.astype(f16))
    common["attn_b"] = colchunk(g("attn_b"), 4)
    common["w_wT"] = kchunk(g("W_W").T.astype(f16))
    common["w_b"] = colchunk(g("W_b"), 8)
    common["copywT"] = colchunk(g("copy_W")[0], 12).astype(f16)
    cb = float(g("copy_b")[0])
    common["copyb_col"] = np.full((128, 1), cb, f32)
    common["copyb_row"] = np.full((1, 1), cb, f32)

    maps = []
    linw_all = g("lin_W")
    linb_all = g("lin_b")
    for c in range(n_cores):
        m = dict(common)
        sl = slice(c * Vs, (c + 1) * Vs)
        m["linwT"] = kchunk(linw_all[sl].T.astype(f16))
        m["linb"] = linb_all[sl].astype(f16)[None, :]
        tl = np.empty((128, SC, NG), f32)
        for gi, (g0, gw, _) in enumerate(vgroups):
            tl[:, :, gi] = (tokens - c * Vs - g0).astype(f32).reshape(SC, 128).T
        m["tokloc_f32"] = np.ascontiguousarray(tl)
        maps.append(m)
    return maps




# ---------------------------------------------------------------------------
# Cached PJRT runner (mirrors bass2jax.run_bass_via_pjrt, but jits/uploads once)
# ---------------------------------------------------------------------------


class Runner:
    def __init__(self, nc, n_cores):
        import jax
        from jax.sharding import Mesh, PartitionSpec
        from jax.experimental.shard_map import shard_map
        from concourse import bass2jax, mybir as _mb
        from concourse.bass2jax import (
            _bass_exec_p, install_neuronx_cc_hook, partition_id_tensor)

        install_neuronx_cc_hook()
        self.nc = nc
        self.n_cores = n_cores
        pname = nc.partition_id_tensor.name if nc.partition_id_tensor else None
        in_names, out_names, out_avals, zero_outs = [], [], [], []
        for alloc in nc.m.functions[0].allocations:
            if not isinstance(alloc, _mb.MemoryLocationSet):
                continue
            name = alloc.memorylocations[0].name
            if alloc.kind == "ExternalInput":
                if name == pname:
                    continue
                in_names.append(name)
            elif alloc.kind == "ExternalOutput":
                shape = tuple(alloc.tensor_shape)
                dtype = _mb.dt.np(alloc.dtype)
                out_names.append(name)
                out_avals.append(jax.core.ShapedArray(shape, dtype))
                zero_outs.append(np.zeros(shape, dtype))
        self.in_names, self.out_names = in_names, out_names
        self.out_avals, self.zero_outs = out_avals, zero_outs
        n_params, n_outs = len(in_names), len(out_names)
        all_names = list(in_names) + list(out_names)
        if pname is not None:
            all_names.append(pname)

        def _body(*args):
            operands = list(args)
            if pname is not None:
                operands.append(partition_id_tensor())
            outs = _bass_exec_p.bind(
                *operands,
                out_avals=tuple(out_avals),
                in_names=tuple(all_names),
                out_names=tuple(out_names),
                lowering_input_output_aliases=(),
                sim_require_finite=True,
                sim_require_nnan=True,
                nc=nc,
            )
            return tuple(outs)

        devices = jax.devices()[:n_cores]
        self.mesh = Mesh(np.asarray(devices), ("core",))
        in_specs = (PartitionSpec("core"),) * (n_params + n_outs)
        out_specs = (PartitionSpec("core"),) * n_outs
        self.fn = jax.jit(
            shard_map(_body, mesh=self.mesh, in_specs=in_specs,
                      out_specs=out_specs, check_rep=False),
            donate_argnums=tuple(range(n_params, n_params + n_outs)),
            keep_unused=True,
        )
        self.jax = jax

    def upload(self, maps):
        import jax
        from jax.sharding import NamedSharding, PartitionSpec
        sh = NamedSharding(self.mesh, PartitionSpec("core"))
        conc = [
            np.concatenate([np.asarray(m[name]) for m in maps], axis=0)
            for name in self.in_names
        ]
        return [jax.device_put(a, sh) for a in conc]

    def zeros(self):
        import jax.numpy as jnp
        from jax.sharding import NamedSharding, PartitionSpec
        sh = NamedSharding(self.mesh, PartitionSpec("core"))
        return [
            jnp.zeros((self.n_cores * z.shape[0], *z.shape[1:]), z.dtype,
                      device=sh)
            for z in self.zero_outs
        ]

    def run(self, dev_inputs):
        outs = self.fn(*dev_inputs, *self.zeros())
        self.jax.block_until_ready(outs)
        return outs

    def results(self, outs):
        r = [dict() for _ in range(self.n_cores)]
        for i, name in enumerate(self.out_names):
            a = np.asarray(outs[i]).reshape(
                self.n_cores, *self.out_avals[i].shape)
            for c in range(self.n_cores):
                r[c][name] = a[c]
        return r


_CACHED = {}


def get_runner():
    if "runner" not in _CACHED:
        nc, _, _ = build_program(
            S_FULL, T_FULL, V // NCORES, NCORES, use_collective=True)
        _CACHED["runner"] = Runner(nc, NCORES)
    return _CACHED["runner"]


def kernel(**inputs):
    maps = prep_inputs(inputs)
    r = get_runner()
    outs = r.run(r.upload(maps))
    res = r.results(outs)
    p_all = np.concatenate(
        [res[c]["p_out"] for c in range(NCORES)], axis=1
    ).astype(np.float32)
    trg = np.asarray(inputs["trg_seqs"]).astype(np.int64)
    p_sel = p_all[np.arange(T_FULL), trg]
    log_prob = np.sum(np.log(p_sel)).astype(np.float32)
    return p_all, log_prob


# revision 27
# speedup vs baseline: 67.2804x; 67.2804x over previous
"""Trainium2 Bass kernel for nn_Decoder (seq2seq LSTM decoder w/ attention + copy).

Strategy: teacher forcing means only the LSTM recurrences are sequential.
 - Encoder fwd/bwd scans interleaved on each core (replicated), fp16 weights,
   orientation-A matvecs (weights stationary) -> gates in [128, nchunk] layout.
 - Decoder scan likewise.
 - Everything else batched: input projections, attention (Q/scores/softmax/ctx),
   output projection, vocab matmul (sharded over V across 8 cores), p_copy via
   onehot matmul, softmax-sum AllReduce, combine, DMA out.
Host: weight layout prep (permute gates, transpose, fp16 cast), final concat +
log-prob reduction.
"""

import numpy as np

import concourse.bass as bass
from concourse import bacc
import concourse.mybir as mybir
import concourse.tile as tile
from concourse.bass import IndirectOffsetOnAxis
from concourse.bass_utils import run_bass_kernel_spmd
from concourse.masks import make_identity

FP = mybir.dt.float32
HP = mybir.dt.float16
I32 = mybir.dt.int32

E = 256
H2 = 512
V = 32000
S_FULL = 512
T_FULL = 128
NCORES = 8
SOS = 1
NEG = -1.0e30
GW = 1024  # vocab psum group width (2 banks of fp32)

AX = mybir.AxisListType
ALU = mybir.AluOpType
ACTF = mybir.ActivationFunctionType


def _perm_ifog(n4: int) -> np.ndarray:
    """Row permutation taking torch gate order [i,f,g,o] -> [i,f,o,g]."""
    h = n4 // 4
    return np.concatenate(
        [np.arange(0, h), np.arange(h, 2 * h), np.arange(3 * h, 4 * h),
         np.arange(2 * h, 3 * h)]
    )


def _vgroups(Vs):
    """PSUM-bank-aligned grouping of the vocab slice: groups of <=1024 cols,
    each split into <=512-wide chunks (one fp32 psum bank per chunk)."""
    groups = []
    g0 = 0
    while g0 < Vs:
        gw = min(GW, Vs - g0)
        chunks = []
        o = 0
        while o < gw:
            w = min(512, gw - o)
            chunks.append((o, w))
            o += w
        groups.append((g0, gw, chunks))
        g0 += gw
    return groups


def build_program(S, T, Vs, n_cores, use_collective, debug=False):
    """Builds the SPMD bass program. Returns (nc, dram_map, debug_names)."""
    nc = bacc.Bacc(
        "TRN2", target_bir_lowering=False, debug=False, num_devices=n_cores
    )
    SC = S // 128
    NG = len(_vgroups(Vs))

    d = {}

    def din(name, shape, dt):
        d[name] = nc.dram_tensor(name, list(shape), dt, kind="ExternalInput")
        return d[name]

    def dout(name, shape, dt):
        d[name] = nc.dram_tensor(name, list(shape), dt, kind="ExternalOutput")
        return d[name]

    din("tokens_i32", (128, SC), I32)
    din("tokens_rev_i32", (128, SC), I32)
    din("trgprev_i32", (128, 1), I32)
    din("tokloc_f32", (128, SC, NG), FP)   # tokens - core*Vs - g*GW
    din("maskvec", (1, S), FP)
    Vtot = Vs * n_cores
    din("enc_emb", (Vtot, E), FP)
    din("dec_emb", (Vtot, H2), FP)
    din("wihT_f", (2, 128, 4 * E), FP)
    din("wihT_b", (2, 128, 4 * E), FP)
    din("whhT_f", (2, 128, 4 * E), HP)
    din("whhT_b", (2, 128, 4 * E), HP)
    din("bias_f", (128, 8), FP)
    din("bias_b", (128, 8), FP)
    din("dwihT_e2", (4, 128, 4 * H2), HP)
    din("dwihT_eh", (4, 128, 4 * H2), HP)
    din("dwhhT", (4, 128, 4 * H2), HP)
    din("dbias", (128, 16), FP)
    din("attn_wT", (4, 128, H2), HP)
    din("attn_b", (128, 4), FP)
    din("w_wT", (8, 128, 2 * H2), HP)
    din("w_b", (128, 8), FP)
    din("linwT", (8, 128, Vs), HP)
    din("linb", (1, Vs), HP)
    din("copywT", (128, 12), HP)
    din("copyb_col", (128, 1), FP)
    din("copyb_row", (1, 1), FP)
    dout("p_out", (128, Vs), FP)

    dbg_names = []

    def ddbg(name, shape, dt=FP):
        if debug:
            dout(name, shape, dt)
            dbg_names.append(name)

    ddbg("dbg_hsf", (128, 2, S), HP)
    ddbg("dbg_hsb", (128, 2, S), HP)
    ddbg("dbg_dh", (128, 4, T), HP)
    ddbg("dbg_aw", (128, S))
    ddbg("dbg_scores", (128, S))
    ddbg("dbg_ctx", (128, 4, 128), HP)
    ddbg("dbg_ohc", (128, 8, 128), HP)
    ddbg("dbg_xwf", (128, 8, S), HP)
    ddbg("dbg_xd", (128, 16, T), HP)
    ddbg("dbg_sexp", (128, 1))
    ddbg("dbg_pg", (128, 1))

    with tile.TileContext(nc) as tc:
        _emit(nc, tc, d, S, T, Vs, SC, n_cores, use_collective, debug)
    nc.compile()
    return nc, d, dbg_names


def _emit(nc, tc, d, S, T, Vs, SC, n_cores, use_collective, debug):
    from contextlib import ExitStack

    vgroups = _vgroups(Vs)
    ctx = ExitStack()
    with ctx:
        sing = ctx.enter_context(tc.tile_pool(name="sing", bufs=1))
        spool = ctx.enter_context(tc.tile_pool(name="scratch", bufs=3))
        dram = ctx.enter_context(tc.tile_pool(name="dramp", bufs=1, space="DRAM"))
        pxw = ExitStack()
        xwpool = pxw.enter_context(tc.tile_pool(name="xwpool", bufs=1))
        pearly = ExitStack()
        early = pearly.enter_context(tc.tile_pool(name="early", bufs=1))
        wpool = pearly.enter_context(tc.tile_pool(name="wstream", bufs=2))
        ppool = pearly.enter_context(
            tc.tile_pool(name="psA", bufs=2, space="PSUM"))

        def st(tag, shape, dt):
            return sing.tile(shape, dt, tag=tag, name=tag)

        def ste(tag, shape, dt):
            return early.tile(shape, dt, tag=tag, name=tag)

        # ---------------- constants / small loads ----------------
        ident_f = st("ident_f", [128, 128], FP)
        make_identity(nc, ident_f[:])
        ident_h = st("ident_h", [128, 128], HP)
        make_identity(nc, ident_h[:])
        ones_h = st("ones_h", [1, 128], HP)
        nc.vector.memset(ones_h[:], 1.0)

        def load(tag, dname, shape, dt, rearr=None):
            t = st(tag, shape, dt)
            src = d[dname].ap()
            if rearr:
                src = src.rearrange(rearr)
            nc.sync.dma_start(out=t[:], in_=src)
            return t

        tok_sb = load("tok_sb", "tokens_i32", [128, SC], I32)
        tokr_sb = load("tokr_sb", "tokens_rev_i32", [128, SC], I32)
        trg_sb = load("trg_sb", "trgprev_i32", [128, 1], I32)
        NGq = len(vgroups)
        tokloc = load("tokloc", "tokloc_f32", [128, SC, NGq], FP)
        maskrow = load("maskrow", "maskvec", [1, S], FP)
        bias_f = load("bias_f", "bias_f", [128, 8], FP)
        bias_b = load("bias_b", "bias_b", [128, 8], FP)
        dbias = load("dbias", "dbias", [128, 16], FP)
        attn_b = load("attn_b", "attn_b", [128, 4], FP)
        w_b = load("w_b", "w_b", [128, 8], FP)
        copywT = load("copywT", "copywT", [128, 12], HP)
        copyb_col = load("copyb_col", "copyb_col", [128, 1], FP)
        copyb_row = load("copyb_row", "copyb_row", [1, 1], FP)
        linb = load("linb", "linb", [1, Vs], HP)
        whh_f = load("whh_f", "whhT_f", [128, 2, 4 * E], HP, "k p m -> p k m")
        whh_b = load("whh_b", "whhT_b", [128, 2, 4 * E], HP, "k p m -> p k m")

        # ---------------- embedding gathers + transposes ----------------
        xT = {}
        for dirn, tsb in (("f", tok_sb), ("b", tokr_sb)):
            xg = ste(f"xgather_{dirn}", [128, SC, E], FP)
            for c in range(SC):
                nc.gpsimd.indirect_dma_start(
                    out=xg[:, c, :], out_offset=None, in_=d["enc_emb"].ap(),
                    in_offset=IndirectOffsetOnAxis(ap=tsb[:, c:c + 1], axis=0),
                )
            xT[dirn] = ste(f"xT_{dirn}", [128, 2, S], FP)
            for c in range(SC):
                for kc in range(2):
                    pt = ppool.tile([128, 128], FP, tag="tp", name="tp")
                    nc.tensor.transpose(
                        pt[:], xg[:, c, kc * 128:(kc + 1) * 128], ident_f[:])
                    nc.scalar.activation(
                        out=xT[dirn][:, kc, c * 128:(c + 1) * 128], in_=pt[:],
                        func=ACTF.Relu)
        e2g = st("e2g", [128, H2], FP)
        nc.gpsimd.indirect_dma_start(
            out=e2g[:, :], out_offset=None, in_=d["dec_emb"].ap(),
            in_offset=IndirectOffsetOnAxis(ap=trg_sb[:, 0:1], axis=0),
        )
        e2T = st("e2T", [128, 4, 128], HP)
        for kc in range(4):
            pt = ppool.tile([128, 128], FP, tag="tp", name="tp")
            nc.tensor.transpose(
                pt[:], e2g[:, kc * 128:(kc + 1) * 128], ident_f[:])
            nc.scalar.activation(out=e2T[:, kc, :], in_=pt[:], func=ACTF.Relu)

        # ---------------- encoder input projections (batched) ----------------
        xwall = xwpool.tile([128, 4, 2, 2, S], HP, tag="xwall", name="xwall")
        for ci, (dirn, wname, bias) in enumerate(
                (("f", "wihT_f", bias_f), ("b", "wihT_b", bias_b))):
            wt = wpool.tile([128, 2, 4 * E], FP, tag="wih", name="wih")
            nc.sync.dma_start(
                out=wt[:], in_=d[wname].ap().rearrange("k p m -> p k m"))
            for j in range(8):
                ps = ppool.tile([128, S], FP, tag="xwps", name="xwps")
                for kc in range(2):
                    nc.tensor.matmul(
                        ps[:], wt[:, kc, j * 128:(j + 1) * 128],
                        xT[dirn][:, kc, :],
                        start=(kc == 0), stop=(kc == 1))
                nc.vector.tensor_scalar(
                    out=xwall[:, j // 2, j % 2, ci, :], in0=ps[:],
                    scalar1=bias[:, j:j + 1], scalar2=None, op0=ALU.add)
        if debug:
            nc.sync.dma_start(out=d["dbg_xwf"].ap(), in_=xwall[:, :, :, 0, :].rearrange("p g s t -> p (g s) t"))

        # ---------------- encoder scans (fwd/bwd interleaved) ----------------
        pearly.close()
        pscan = ExitStack()
        gpool_f = pscan.enter_context(
            tc.tile_pool(name="gates_f", bufs=2, space="PSUM"))
        gpool_b = pscan.enter_context(
            tc.tile_pool(name="gates_b", bufs=2, space="PSUM"))
        hs = {"f": st("hs_f", [128, 2, S], HP), "b": st("hs_b", [128, 2, S], HP)}
        h0 = st("h0", [128, 4], HP)
        nc.vector.memset(h0[:], 0.0)
        ctg_prev = spool.tile([128, 4, 2], FP, tag="ctge", name="ctge")
        nc.vector.memset(ctg_prev[:], 0.0)
        # gboth psum layout [128, gate(4: i,f,o,g), chain(2: f,b), sub(2)]
        # column order within gboth: col = 4*gate + 2*sub + chain
        # gates: 0=i 1=f 2=o 3=g ; sub = hidden chunk ; chain: 0=f 1=b
        hprev = h0
        for t in range(S):
            ps = gpool_f.tile([128, 4, 2, 2], FP, tag="gb", name="gb")
            nc.tensor.matmul(
                ps[:], ident_h[:], xwall[:, :, :, :, t],
                start=True, stop=False, skip_group_check=True)
            for ci, whh_sb in ((0, whh_f), (1, whh_b)):
                for j in range(8):
                    for kc in range(2):
                        rhs = hprev[:, 2 * kc + ci:2 * kc + ci + 1]
                        nc.tensor.matmul(
                            ps[:, j // 2, j % 2, ci:ci + 1],
                            whh_sb[:, kc, j * 128:(j + 1) * 128], rhs,
                            start=False, stop=(kc == 1),
                            skip_group_check=True)
            # pair tile: [:, k, 0] = tanh(g_t) (this step),
            # [:, k, 1] = c_{t-1} (written last step) -> prod+reduce -> c_t
            ctg_new = spool.tile([128, 4, 2], FP, tag="ctge", name="ctge")
            nc.scalar.activation(out=ctg_prev[:, :, 0], in_=ps[:, 3, :, :],
                                 func=ACTF.Tanh)
            sg = spool.tile([128, 12], FP, tag="sge", name="sge")
            nc.scalar.activation(out=sg[:], in_=ps[:, 0:3, :, :],
                                 func=ACTF.Sigmoid)
            # prod[:, k, 0] = i*tg ; prod[:, k, 1] = f*c  -> reduce X -> c_new
            prod = spool.tile([128, 4, 2], FP, tag="prde", name="prde")
            nc.vector.tensor_tensor(
                out=prod[:],
                in0=sg[:, 0:8].rearrange("p (x k) -> p k x", x=2),
                in1=ctg_prev[:], op=ALU.mult)
            nc.vector.tensor_reduce(
                out=ctg_new[:, :, 1], in_=prod[:], axis=AX.X, op=ALU.add)
            tc_ = spool.tile([128, 4], FP, tag="tce", name="tce")
            nc.scalar.activation(out=tc_[:], in_=ctg_new[:, :, 1],
                                 func=ACTF.Tanh)
            hnew = spool.tile([128, 4], HP, tag="hne", name="hne")
            nc.vector.tensor_tensor(
                out=hnew[:], in0=sg[:, 8:12], in1=tc_[:], op=ALU.mult)
            hv = hnew[:].rearrange("p (s c) -> p s c", c=2)
            nc.gpsimd.tensor_copy(out=hs["f"][:, :, t], in_=hv[:, :, 0])
            nc.gpsimd.tensor_copy(
                out=hs["b"][:, :, S - 1 - t], in_=hv[:, :, 1])
            hprev = hnew
            ctg_prev = ctg_new
        if debug:
            nc.sync.dma_start(out=d["dbg_hsf"].ap(), in_=hs["f"][:])
            nc.sync.dma_start(out=d["dbg_hsb"].ap(), in_=hs["b"][:])

        def ehid_chunk(kc):
            if kc < 2:
                return hs["f"][:, kc, S - 1:S]
            return hs["b"][:, kc - 2, 0:1]

        # ---------------- decoder input precompute ----------------
        pscan.close()
        pxw.close()
        late = ctx.enter_context(tc.tile_pool(name="late", bufs=1))
        wpool2 = ctx.enter_context(tc.tile_pool(name="wstream2", bufs=2))
        sm1 = ctx.enter_context(tc.tile_pool(name="sm1", bufs=1))
        ohpool = ctx.enter_context(tc.tile_pool(name="ohpool", bufs=2))

        def stl(tag, shape, dt):
            return late.tile(shape, dt, tag=tag, name=tag)

        dwhh = stl("dwhh", [128, 4, 4 * H2], HP)
        nc.sync.dma_start(
            out=dwhh[:], in_=d["dwhhT"].ap().rearrange("k p m -> p k m"))
        linw = stl("linw", [128, 8, Vs], HP)
        nc.sync.dma_start(
            out=linw[:], in_=d["linwT"].ap().rearrange("k p m -> p k m"))
        iota = stl("iota", [128, GW], FP)
        nc.gpsimd.iota(
            out=iota[:], pattern=[[1, GW]], base=0, channel_multiplier=0,
            allow_small_or_imprecise_dtypes=True,
        )
        pb2 = ExitStack()
        bigp = pb2.enter_context(
            tc.tile_pool(name="psB2", bufs=1, space="PSUM"))
        ppool2 = pb2.enter_context(
            tc.tile_pool(name="psB2s", bufs=1, space="PSUM"))
        xd_ps = bigp.tile([128, 16, T], FP, tag="xdps", name="xdps")
        we2 = wpool2.tile([128, 4, 4 * H2], HP, tag="wdec", name="we2")
        nc.sync.dma_start(
            out=we2[:], in_=d["dwihT_e2"].ap().rearrange("k p m -> p k m"))
        for j in range(16):
            for kc in range(4):
                nc.tensor.matmul(
                    xd_ps[:, j, :], we2[:, kc, j * 128:(j + 1) * 128],
                    e2T[:, kc, :], start=(kc == 0), stop=(kc == 3))
        cvec_ps = ppool2.tile([128, 16], FP, tag="cvps", name="cvps")
        weh = wpool2.tile([128, 4, 4 * H2], HP, tag="wdec", name="weh")
        nc.sync.dma_start(
            out=weh[:], in_=d["dwihT_eh"].ap().rearrange("k p m -> p k m"))
        for j in range(16):
            for kc in range(4):
                nc.tensor.matmul(
                    cvec_ps[:, j:j + 1], weh[:, kc, j * 128:(j + 1) * 128],
                    ehid_chunk(kc), start=(kc == 0), stop=(kc == 3))
        cvec = stl("cvec", [128, 16], FP)
        nc.vector.tensor_tensor(
            out=cvec[:], in0=cvec_ps[:], in1=dbias[:], op=ALU.add)
        xd = stl("xd", [128, 16, T], HP)
        for j in range(16):
            nc.vector.tensor_scalar(
                out=xd[:, j, :], in0=xd_ps[:, j, :],
                scalar1=cvec[:, j:j + 1], scalar2=None, op0=ALU.add)
        if debug:
            nc.sync.dma_start(out=d["dbg_xd"].ap(), in_=xd[:])

        # ---------------- decoder scan ----------------
        pb2.close()
        pdec = ExitStack()
        gpool_d = pdec.enter_context(
            tc.tile_pool(name="gates_d", bufs=2, space="PSUM"))
        dh = st("dh", [128, 4, T], HP)
        dctg_prev = spool.tile([128, 4, 2], FP, tag="ctgd", name="ctgd")
        nc.vector.memset(dctg_prev[:], 0.0)
        for t in range(T):
            ps = gpool_d.tile([128, 16], FP, tag="gd", name="gd")
            nc.tensor.matmul(
                ps[:], ident_h[:], xd[:, :, t],
                start=True, stop=False, skip_group_check=True)
            for j in range(16):
                for kc in range(4):
                    rhs = ehid_chunk(kc) if t == 0 else dhprev[:, kc:kc + 1]
                    nc.tensor.matmul(
                        ps[:, j:j + 1],
                        dwhh[:, kc, j * 128:(j + 1) * 128], rhs,
                        start=False, stop=(kc == 3),
                        skip_group_check=True)
            dctg_new = spool.tile([128, 4, 2], FP, tag="ctgd", name="ctgd")
            nc.scalar.activation(out=dctg_prev[:, :, 0], in_=ps[:, 12:16],
                                 func=ACTF.Tanh)
            sg = spool.tile([128, 12], FP, tag="sgd", name="sgd")
            nc.scalar.activation(out=sg[:], in_=ps[:, 0:12], func=ACTF.Sigmoid)
            prod = spool.tile([128, 4, 2], FP, tag="prdd", name="prdd")
            nc.vector.tensor_tensor(
                out=prod[:],
                in0=sg[:, 0:8].rearrange("p (x k) -> p k x", x=2),
                in1=dctg_prev[:], op=ALU.mult)
            nc.vector.tensor_reduce(
                out=dctg_new[:, :, 1], in_=prod[:], axis=AX.X, op=ALU.add)
            tc_ = spool.tile([128, 4], FP, tag="tcd", name="tcd")
            nc.scalar.activation(out=tc_[:], in_=dctg_new[:, :, 1],
                                 func=ACTF.Tanh)
            dhnew = spool.tile([128, 4], HP, tag="dhn", name="dhn")
            nc.vector.tensor_tensor(
                out=dhnew[:], in0=sg[:, 8:12], in1=tc_[:], op=ALU.mult)
            nc.gpsimd.tensor_copy(out=dh[:, :, t], in_=dhnew[:])
            dhprev = dhnew
            dctg_prev = dctg_new
        if debug:
            nc.sync.dma_start(out=d["dbg_dh"].ap(), in_=dh[:])

        # ---------------- batched attention ----------------
        pdec.close()
        patt = ExitStack()
        ppool = patt.enter_context(
            tc.tile_pool(name="psD", bufs=2, space="PSUM"))
        ppd1 = patt.enter_context(
            tc.tile_pool(name="psD1", bufs=1, space="PSUM"))
        qt = st("qt", [128, 4, T], HP)
        wat = wpool2.tile([128, 4, H2], HP, tag="wdec", name="wat")
        nc.sync.dma_start(
            out=wat[:], in_=d["attn_wT"].ap().rearrange("k p m -> p k m"))
        for jq in range(4):
            ps = ppool.tile([128, T], FP, tag="mm128", name="mm128")
            for kc in range(4):
                nc.tensor.matmul(
                    ps[:], wat[:, kc, jq * 128:(jq + 1) * 128], dh[:, kc, :],
                    start=(kc == 0), stop=(kc == 3))
            nc.vector.tensor_scalar(
                out=qt[:, jq, :], in0=ps[:], scalar1=attn_b[:, jq:jq + 1],
                scalar2=None, op0=ALU.add)

        def encT_chunk(kc):
            return hs["f"][:, kc, :] if kc < 2 else hs["b"][:, kc - 2, :]

        sc_ps = ppd1.tile([128, S], FP, tag="scps", name="scps")
        for kc in range(4):
            nc.tensor.matmul(
                sc_ps[:], qt[:, kc, :], encT_chunk(kc),
                start=(kc == 0), stop=(kc == 3))
        mb128 = sm1.tile([128, S], FP, tag="mb128", name="mb128")
        mv_ap = d["maskvec"].ap()
        nc.sync.dma_start(
            out=mb128[:],
            in_=bass.AP(tensor=mv_ap.tensor, offset=mv_ap.offset,
                        ap=[[0, 128]] + [list(p) for p in mv_ap.ap[1:]]))
        scores = sm1.tile([128, S], FP, tag="scores", name="scores")
        nc.vector.tensor_tensor(
            out=scores[:], in0=sc_ps[:], in1=mb128[:], op=ALU.add)
        if debug:
            nc.sync.dma_start(out=d["dbg_scores"].ap(), in_=scores[:])
        rmax = sm1.tile([128, 1], FP, tag="rmax", name="rmax")
        nc.vector.tensor_reduce(out=rmax[:], in_=scores[:], axis=AX.X, op=ALU.max)
        nmax = sm1.tile([128, 1], FP, tag="nmax", name="nmax")
        nc.scalar.activation(out=nmax[:], in_=rmax[:], func=ACTF.Copy, scale=-1.0)
        aexp = sm1.tile([128, S], FP, tag="aexp", name="aexp")
        sexp = sm1.tile([128, 1], FP, tag="sexp", name="sexp")
        nc.scalar.activation(
            out=aexp[:], in_=scores[:], func=ACTF.Exp, bias=nmax[:, 0:1],
            accum_out=sexp[:, 0:1])
        rsum = sm1.tile([128, 1], FP, tag="rsum", name="rsum")
        nc.vector.reciprocal(out=rsum[:], in_=sexp[:])
        aw16 = sm1.tile([128, S], HP, tag="aw16", name="aw16")
        nc.vector.tensor_scalar(
            out=aw16[:], in0=aexp[:], scalar1=rsum[:, 0:1], scalar2=None,
            op0=ALU.mult)
        if debug:
            awdbg = sm1.tile([128, S], FP, tag="awdbg", name="awdbg")
            nc.vector.tensor_scalar(
                out=awdbg[:], in0=aexp[:], scalar1=rsum[:, 0:1], scalar2=None,
                op0=ALU.mult)
            nc.sync.dma_start(out=d["dbg_aw"].ap(), in_=awdbg[:])

        awt = stl("awt", [128, SC, T], HP)
        for c in range(SC):
            pt = ppool.tile([128, 128], HP, tag="tph", name="tph")
            nc.tensor.transpose(
                pt[:], aw16[:, c * 128:(c + 1) * 128], ident_h[:])
            nc.scalar.copy(out=awt[:, c, :], in_=pt[:, 0:T])
        encs = stl("encs", [128, SC, 4, 128], HP)
        for c in range(SC):
            for kd in range(4):
                pt = ppool.tile([128, 128], HP, tag="tph", name="tph")
                nc.tensor.transpose(
                    pt[:], encT_chunk(kd)[:, c * 128:(c + 1) * 128], ident_h[:])
                nc.scalar.copy(out=encs[:, c, kd, :], in_=pt[:])
        ctx16 = stl("ctx16", [128, 4, T], HP)
        for kd in range(4):
            ps = ppool.tile([128, T], FP, tag="mm128", name="mm128")
            for c in range(SC):
                nc.tensor.matmul(
                    ps[:], encs[:, c, kd, :], awt[:, c, :],
                    start=(c == 0), stop=(c == SC - 1))
            nc.scalar.copy(out=ctx16[:, kd, :], in_=ps[:])
        if debug:
            nc.sync.dma_start(out=d["dbg_ctx"].ap(), in_=ctx16[:])

        def hcT_chunk(kc):
            return dh[:, kc, :] if kc < 4 else ctx16[:, kc - 4, :]

        # ---------------- output projection OUT_HC.T ----------------
        ohc = stl("ohc", [128, 8, T], HP)
        wwt = stl("wwt", [128, 8, 2 * H2], HP)
        nc.sync.dma_start(
            out=wwt[:], in_=d["w_wT"].ap().rearrange("k p m -> p k m"))
        for jo in range(8):
            ps = ppool.tile([128, T], FP, tag="mm128", name="mm128")
            for kc in range(8):
                nc.tensor.matmul(
                    ps[:], wwt[:, kc, jo * 128:(jo + 1) * 128],
                    hcT_chunk(kc), start=(kc == 0), stop=(kc == 7))
            nc.vector.tensor_scalar(
                out=ohc[:, jo, :], in0=ps[:], scalar1=w_b[:, jo:jo + 1],
                scalar2=None, op0=ALU.add)
        if debug:
            nc.sync.dma_start(out=d["dbg_ohc"].ap(), in_=ohc[:])

        # ---------------- p_gen ----------------
        def hcxT_chunk(kc):
            if kc < 4:
                return dh[:, kc, :]
            if kc < 8:
                return ctx16[:, kc - 4, :]
            return e2T[:, kc - 8, :]

        pg_ps = ppd1.tile([128, 1], FP, tag="pgps", name="pgps")
        for kc in range(12):
            nc.tensor.matmul(
                pg_ps[:], hcxT_chunk(kc), copywT[:, kc:kc + 1],
                start=(kc == 0), stop=(kc == 11))
        pgr_ps = ppd1.tile([1, T], FP, tag="pgrps", name="pgrps")
        for kc in range(12):
            nc.tensor.matmul(
                pgr_ps[:], copywT[:, kc:kc + 1], hcxT_chunk(kc),
                start=(kc == 0), stop=(kc == 11))
        pg_col = stl("pg_col", [128, 1], FP)
        nc.scalar.activation(
            out=pg_col[:], in_=pg_ps[:], func=ACTF.Sigmoid,
            bias=copyb_col[:, 0:1])
        pg_row = stl("pg_row", [1, T], FP)
        nc.scalar.activation(
            out=pg_row[:], in_=pgr_ps[:], func=ACTF.Sigmoid,
            bias=copyb_row[:, 0:1])
        ompg_row = stl("ompg_row", [1, T], HP)
        nc.scalar.activation(
            out=ompg_row[:], in_=pg_row[:], func=ACTF.Copy, scale=-1.0, bias=1.0)
        omb_ps = ppool.tile([128, T], FP, tag="mm128", name="mm128")
        nc.tensor.matmul(omb_ps[:], ones_h[:], ompg_row[:],
                         start=True, stop=True)
        awts = stl("awts", [128, SC, T], HP)
        for c in range(SC):
            nc.vector.tensor_tensor(
                out=awts[:, c, :], in0=awt[:, c, :],
                in1=omb_ps[:], op=ALU.mult)
        if debug:
            nc.sync.dma_start(out=d["dbg_pg"].ap(), in_=pg_col[:])

        # ---------------- vocab logits + exp + sums ----------------
        patt.close()
        pend = ExitStack()
        bigp = pend.enter_context(
            tc.tile_pool(name="psE", bufs=1, space="PSUM"))
        ebuf = stl("ebuf", [128, Vs], FP)
        nparts = sum(len(g[2]) for g in vgroups)
        sparts = stl("sparts", [128, nparts], FP)
        pi = 0
        for g0, gw, chunks in vgroups:
            ps = bigp.tile([128, 2, 512], FP, tag="lgps", name="lgps", bufs=2)
            for ci, (o, w) in enumerate(chunks):
                for kc in range(8):
                    nc.tensor.matmul(
                        ps[:, ci, 0:w], ohc[:, kc, :],
                        linw[:, kc, g0 + o:g0 + o + w],
                        start=(kc == 0), stop=False)
                nc.tensor.matmul(
                    ps[:, ci, 0:w], ones_h[:], linb[:, g0 + o:g0 + o + w],
                    start=False, stop=True)
                nc.scalar.activation(
                    out=ebuf[:, g0 + o:g0 + o + w], in_=ps[:, ci, 0:w],
                    func=ACTF.Exp, accum_out=sparts[:, pi:pi + 1])
                pi += 1
        sloc = stl("sloc", [128, 1], FP)
        nc.vector.tensor_reduce(
            out=sloc[:], in_=sparts[:], axis=AX.X, op=ALU.add)

        # ---------------- softmax-sum allreduce ----------------
        sg_t = stl("sg_t", [128, 1], FP)
        if use_collective and n_cores > 1:
            cin = dram.tile([128, 1], FP, tag="ccin", name="ccin")
            cout = dram.tile([128, 1], FP, tag="ccout", name="ccout")
            nc.sync.dma_start(out=cin[:], in_=sloc[:])
            nc.gpsimd.collective_compute(
                "AllReduce", ALU.add,
                replica_groups=[list(range(n_cores))],
                ins=[cin[:].opt()], outs=[cout[:].opt()],
            )
            nc.sync.dma_start(out=sg_t[:], in_=cout[:])
        else:
            nc.vector.tensor_copy(out=sg_t[:], in_=sloc[:])
        if debug:
            nc.sync.dma_start(out=d["dbg_sexp"].ap(), in_=sg_t[:])

        rg = stl("rg", [128, 1], FP)
        nc.vector.reciprocal(out=rg[:], in_=sg_t[:])
        pgs = stl("pgs", [128, 1], FP)
        nc.vector.tensor_tensor(out=pgs[:], in0=pg_col[:], in1=rg[:], op=ALU.mult)
        nc.vector.tensor_scalar(
            out=ebuf[:], in0=ebuf[:], scalar1=pgs[:, 0:1], scalar2=None,
            op0=ALU.mult)

        # ---------------- p_copy = AW.T(1-pg) @ onehot ----------------
        for gi, (g0, gw, chunks) in enumerate(vgroups):
            ps = bigp.tile([128, 2, 512], FP, tag="pcps", name="pcps", bufs=2)
            for c in range(SC):
                oh = ohpool.tile([128, GW], HP, tag="oh", name="oh")
                nc.gpsimd.tensor_scalar(
                    out=oh[:, 0:gw], in0=iota[:, 0:gw],
                    scalar1=tokloc[:, c, gi:gi + 1], scalar2=None,
                    op0=ALU.is_equal)
                for ci, (o, w) in enumerate(chunks):
                    nc.tensor.matmul(
                        ps[:, ci, 0:w], awts[:, c, :], oh[:, o:o + w],
                        start=(c == 0), stop=(c == SC - 1))
            for ci, (o, w) in enumerate(chunks):
                nc.vector.tensor_tensor(
                    out=ebuf[:, g0 + o:g0 + o + w],
                    in0=ebuf[:, g0 + o:g0 + o + w],
                    in1=ps[:, ci, 0:w], op=ALU.add)
        nc.sync.dma_start(out=d["p_out"].ap(), in_=ebuf[:])
        pend.close()


# ---------------------------------------------------------------------------
# Host-side preparation
# ---------------------------------------------------------------------------


def prep_inputs(inputs, n_cores=NCORES, S=S_FULL, T=T_FULL, Vtot=V):
    f32, f16 = np.float32, np.float16
    Vs = Vtot // n_cores
    SC = S // 128
    vgroups = _vgroups(Vs)
    NG = len(vgroups)

    tokens = np.asarray(inputs["tokens"]).astype(np.int64)
    trg = np.asarray(inputs["trg_seqs"]).astype(np.int64)
    trgprev = np.concatenate([[SOS], trg[:-1]]).astype(np.int32)

    perm_e = _perm_ifog(4 * E)
    perm_d = _perm_ifog(4 * H2)

    def kchunk(a):
        K, M = a.shape
        return np.ascontiguousarray(a.reshape(K // 128, 128, M))

    def colchunk(v, nch):
        return np.ascontiguousarray(v.reshape(nch, 128).T)

    def g(name):
        return np.asarray(inputs[name]).astype(f32)

    common = {}
    common["tokens_i32"] = np.ascontiguousarray(
        tokens.astype(np.int32).reshape(SC, 128).T)
    common["tokens_rev_i32"] = np.ascontiguousarray(
        tokens[::-1].astype(np.int32).reshape(SC, 128).T)
    common["trgprev_i32"] = trgprev.reshape(1, T).T.copy()
    common["maskvec"] = np.where(tokens > 0, 0.0, NEG).astype(f32)[None, :]
    common["enc_emb"] = g("enc_emb")
    common["dec_emb"] = g("dec_emb")
    for dirn in ("f", "b"):
        wih = g(f"Wih_{dirn}")[perm_e]
        whh = g(f"Whh_{dirn}")[perm_e]
        bias = (g(f"bih_{dirn}") + g(f"bhh_{dirn}"))[perm_e]
        common[f"wihT_{dirn}"] = kchunk(wih.T.astype(f32))
        common[f"whhT_{dirn}"] = kchunk(whh.T.astype(f16))
        common[f"bias_{dirn}"] = colchunk(bias, 8)
    dwih = g("dWih")[perm_d]
    dwhh = g("dWhh")[perm_d]
    dbias = (g("dbih") + g("dbhh"))[perm_d]
    common["dwihT_e2"] = kchunk(dwih[:, :H2].T.astype(f16))
    common["dwihT_eh"] = kchunk(dwih[:, H2:].T.astype(f16))
    common["dwhhT"] = kchunk(dwhh.T.astype(f16))
    common["dbias"] = colchunk(dbias, 16)
    common["attn_wT"] = kchunk(g("attn_W").T.astype(f16))
    common["attn_b"] = colchunk(g("attn_b"), 4)
    common["w_wT"] = kchunk(g("W_W").T.astype(f16))
    common["w_b"] = colchunk(g("W_b"), 8)
    common["copywT"] = colchunk(g("copy_W")[0], 12).astype(f16)
    cb = float(g("copy_b")[0])
    common["copyb_col"] = np.full((128, 1), cb, f32)
    common["copyb_row"] = np.full((1, 1), cb, f32)

    maps = []
    linw_all = g("lin_W")
    linb_all = g("lin_b")
    for c in range(n_cores):
        m = dict(common)
        sl = slice(c * Vs, (c + 1) * Vs)
        m["linwT"] = kchunk(linw_all[sl].T.astype(f16))
        m["linb"] = linb_all[sl].astype(f16)[None, :]
        tl = np.empty((128, SC, NG), f32)
        for gi, (g0, gw, _) in enumerate(vgroups):
            tl[:, :, gi] = (tokens - c * Vs - g0).astype(f32).reshape(SC, 128).T
        m["tokloc_f32"] = np.ascontiguousarray(tl)
        maps.append(m)
    return maps




# ---------------------------------------------------------------------------
# Cached PJRT runner (mirrors bass2jax.run_bass_via_pjrt, but jits/uploads once)
# ---------------------------------------------------------------------------


class Runner:
    def __init__(self, nc, n_cores):
        import jax
        from jax.sharding import Mesh, PartitionSpec
        from jax.experimental.shard_map import shard_map
        from concourse import bass2jax, mybir as _mb
        from concourse.bass2jax import (
            _bass_exec_p, install_neuronx_cc_hook, partition_id_tensor)

        install_neuronx_cc_hook()
        self.nc = nc
        self.n_cores = n_cores
        pname = nc.partition_id_tensor.name if nc.partition_id_tensor else None
        in_names, out_names, out_avals, zero_outs = [], [], [], []
        for alloc in nc.m.functions[0].allocations:
            if not isinstance(alloc, _mb.MemoryLocationSet):
                continue
            name = alloc.memorylocations[0].name
            if alloc.kind == "ExternalInput":
                if name == pname:
                    continue
                in_names.append(name)
            elif alloc.kind == "ExternalOutput":
                shape = tuple(alloc.tensor_shape)
                dtype = _mb.dt.np(alloc.dtype)
                out_names.append(name)
                out_avals.append(jax.core.ShapedArray(shape, dtype))
                zero_outs.append(np.zeros(shape, dtype))
        self.in_names, self.out_names = in_names, out_names
        self.out_avals, self.zero_outs = out_avals, zero_outs
        n_params, n_outs = len(in_names), len(out_names)
        all_names = list(in_names) + list(out_names)
        if pname is not None:
            all_names.append(pname)

        def _body1(*args):
            operands = list(args)
            if pname is not None:
                operands.append(partition_id_tensor())
            outs = _bass_exec_p.bind(
                *operands,
                out_avals=tuple(out_avals),
                in_names=tuple(all_names),
                out_names=tuple(out_names),
                lowering_input_output_aliases=(),
                sim_require_finite=True,
                sim_require_nnan=True,
                nc=nc,
            )
            return tuple(outs)

        def make_body(k):
            def _body(*args):
                ins = list(args[:n_params])
                outs = list(args[n_params:])
                for _ in range(k):
                    outs = list(_body1(*ins, *outs))
                return tuple(outs)
            return _body
        self.make_body = make_body
        _body = make_body(1)

        devices = jax.devices()[:n_cores]
        self.mesh = Mesh(np.asarray(devices), ("core",))
        in_specs = (PartitionSpec("core"),) * (n_params + n_outs)
        out_specs = (PartitionSpec("core"),) * n_outs
        self._mkjit = lambda body: jax.jit(
            shard_map(body, mesh=self.mesh, in_specs=in_specs,
                      out_specs=out_specs, check_rep=False),
            donate_argnums=tuple(range(n_params, n_params + n_outs)),
            keep_unused=True,
        )
        self.fn = self._mkjit(_body)
        self._fnk = {1: self.fn}
        self.jax = jax

    def upload(self, maps):
        import jax
        from jax.sharding import NamedSharding, PartitionSpec
        sh = NamedSharding(self.mesh, PartitionSpec("core"))
        conc = [
            np.concatenate([np.asarray(m[name]) for m in maps], axis=0)
            for name in self.in_names
        ]
        return [jax.device_put(a, sh) for a in conc]

    def zeros(self):
        import jax.numpy as jnp
        from jax.sharding import NamedSharding, PartitionSpec
        sh = NamedSharding(self.mesh, PartitionSpec("core"))
        return [
            jnp.zeros((self.n_cores * z.shape[0], *z.shape[1:]), z.dtype,
                      device=sh)
            for z in self.zero_outs
        ]

    def run(self, dev_inputs, k=1):
        if k not in self._fnk:
            self._fnk[k] = self._mkjit(self.make_body(k))
        outs = self._fnk[k](*dev_inputs, *self.zeros())
        self.jax.block_until_ready(outs)
        return outs

    def results(self, outs):
        r = [dict() for _ in range(self.n_cores)]
        for i, name in enumerate(self.out_names):
            a = np.asarray(outs[i]).reshape(
                self.n_cores, *self.out_avals[i].shape)
            for c in range(self.n_cores):
                r[c][name] = a[c]
        return r


_CACHED = {}


def get_runner():
    if "runner" not in _CACHED:
        nc, _, _ = build_program(
            S_FULL, T_FULL, V // NCORES, NCORES, use_collective=True)
        _CACHED["runner"] = Runner(nc, NCORES)
    return _CACHED["runner"]


def kernel(**inputs):
    maps = prep_inputs(inputs)
    r = get_runner()
    outs = r.run(r.upload(maps))
    res = r.results(outs)
    p_all = np.concatenate(
        [res[c]["p_out"] for c in range(NCORES)], axis=1
    ).astype(np.float32)
    trg = np.asarray(inputs["trg_seqs"]).astype(np.int64)
    p_sel = p_all[np.arange(T_FULL), trg]
    log_prob = np.sum(np.log(p_sel)).astype(np.float32)
    return p_all, log_prob
